# revision 1
# baseline (speedup 1.0000x reference)
"""v1.5: bf16 broadcasts + bf16 bb/p muls, PE identity-matmul accumulation
over the 16 states, f32r projections. See kernel.py for the base design."""
import sys
sys.path.insert(0, "/opt/trn_rl_repo")
import numpy as np

B_GLOB = 16
N_CORES = 8
B_LOC = B_GLOB // N_CORES
L = 4096
T = 512
NCH = L // T
DST = 16
DIN = 256
DH = DIN // 128

_BUILT = {}
H_BF16 = True   # variant B: scan output in bf16


def build_module(reps=1):
    import concourse.bass as bass
    import concourse.tile as tile
    from concourse import bacc, mybir

    F32 = mybir.dt.float32
    F32R = mybir.dt.float32r
    BF16 = mybir.dt.bfloat16
    ALU = mybir.AluOpType
    ACTF = mybir.ActivationFunctionType
    from concourse.ap import AP

    nc = bacc.Bacc("TRN2", target_bir_lowering=False, debug=False,
                   num_devices=N_CORES)

    x_d = nc.dram_tensor("x", [B_LOC, 128, L], F32R, kind="ExternalInput")
    w2k_d = nc.dram_tensor("w2k", [4, 128, 256], F32R, kind="ExternalInput")
    winz_d = nc.dram_tensor("winz", [128, 256], F32R, kind="ExternalInput")
    wxp_d = nc.dram_tensor("wxp", [2, 128, 40], F32R, kind="ExternalInput")
    wdt_d = nc.dram_tensor("wdt", [8, 256], F32R, kind="ExternalInput")
    wout_d = nc.dram_tensor("wout", [2, 128, 128], F32R, kind="ExternalInput")
    cb_d = nc.dram_tensor("cb", [128, 2], F32, kind="ExternalInput")
    bdt_d = nc.dram_tensor("bdt", [128, 2], F32, kind="ExternalInput")
    dpar_d = nc.dram_tensor("dpar", [128, 2], F32, kind="ExternalInput")
    acols_d = nc.dram_tensor("acols", [128, 32], F32, kind="ExternalInput")
    ident_d = nc.dram_tensor("ident", [128, 128], BF16, kind="ExternalInput")
    out_d = nc.dram_tensor("out", [B_LOC, 128, 64, 64], F32,
                           kind="ExternalOutput")

    HDT = BF16 if H_BF16 else F32

    with tile.TileContext(nc) as tc:
        with (
            tc.tile_pool(name="consts", bufs=1) as consts,
            tc.tile_pool(name="big", bufs=1) as big,
            tc.tile_pool(name="ld", bufs=3) as ld,
            tc.tile_pool(name="sb", bufs=2) as sb,
            tc.tile_pool(name="sc", bufs=4) as sc,
            tc.tile_pool(name="bc", bufs=12) as bcp,
            tc.tile_pool(name="psum", bufs=2, space=bass.MemorySpace.PSUM) as psum,
            tc.tile_pool(name="dram", bufs=18, space=bass.MemorySpace.DRAM) as dram,
        ):
            w2k_t = consts.tile([128, 4 * 256], F32R)
            for k in range(4):
                nc.sync.dma_start(w2k_t[:, k * 256:(k + 1) * 256], w2k_d.ap()[k])
            winz_t = consts.tile([128, 256], F32R)
            nc.sync.dma_start(winz_t[:], winz_d.ap())
            wxp_t = consts.tile([128, 80], F32R)
            for j in range(2):
                nc.sync.dma_start(wxp_t[:, j * 40:(j + 1) * 40], wxp_d.ap()[j])
            wdt_t = consts.tile([8, 256], F32R)
            nc.sync.dma_start(wdt_t[:], wdt_d.ap())
            wout_t = consts.tile([128, 256], F32R)
            for j in range(2):
                nc.sync.dma_start(wout_t[:, j * 128:(j + 1) * 128], wout_d.ap()[j])
            cb_t = consts.tile([128, 2], F32)
            nc.sync.dma_start(cb_t[:], cb_d.ap())
            bdt_t = consts.tile([128, 2], F32)
            nc.sync.dma_start(bdt_t[:], bdt_d.ap())
            dpar_t = consts.tile([128, 2], F32)
            nc.sync.dma_start(dpar_t[:], dpar_d.ap())
            acols_t = consts.tile([128, 32], F32)
            nc.sync.dma_start(acols_t[:], acols_d.ap())
            ident_t = consts.tile([128, 128], BF16)
            nc.sync.dma_start(ident_t[:], ident_d.ap())

            yg = [big.tile([128, L], F32R, tag=f"yg{dh}", name=f"yg{dh}")
                  for dh in range(DH)]
            xcf = [big.tile([128, L], F32, tag=f"xcf{dh}", name=f"xcf{dh}")
                   for dh in range(DH)]
            szf = [big.tile([128, L], F32, tag=f"szf{dh}", name=f"szf{dh}")
                   for dh in range(DH)]

            for rep in range(reps):
                for b in range(B_LOC):
                    bcds = []

                    def emit_A(ci, b=b, bcds=None):
                        bcds = _bcds
                        t0 = ci * T
                        xck = ld.tile([128, T + 3], F32R, tag="xck", name="xck")
                        if ci == 0:
                            nc.gpsimd.memset(xck[:, 0:3].bitcast(F32), 0.0)
                            nc.sync.dma_start(xck[:, 3:], x_d.ap()[b][:, 0:T])
                        else:
                            nc.sync.dma_start(xck[:],
                                              x_d.ap()[b][:, t0 - 3:t0 + T])
                        for dh in range(DH):
                            xcps = psum.tile([128, T], F32, tag="xcps", bufs=1,
                                             name="xcps")
                            for k in range(4):
                                nc.tensor.matmul(
                                    xcps[:],
                                    w2k_t[:, k * 256 + dh * 128:
                                          k * 256 + (dh + 1) * 128],
                                    xck[:, k:k + T],
                                    start=(k == 0), stop=(k == 3))
                            nc.scalar.activation(xcf[dh][:, t0:t0 + T], xcps[:],
                                                 ACTF.Silu,
                                                 bias=cb_t[:, dh:dh + 1])
                            zps = psum.tile([128, T], F32, tag="zps", bufs=1,
                                            name="zps")
                            nc.tensor.matmul(
                                zps[:],
                                winz_t[:, dh * 128:(dh + 1) * 128],
                                xck[:, 3:3 + T], start=True, stop=True)
                            nc.scalar.activation(szf[dh][:, t0:t0 + T], zps[:],
                                                 ACTF.Silu)
                        xcr = ld.tile([128, 2 * T], F32R, tag="xcr", name="xcr")
                        for dh in range(DH):
                            nc.scalar.copy(xcr[:, dh * T:(dh + 1) * T],
                                           xcf[dh][:, t0:t0 + T])
                        xpps = psum.tile([40, T], F32, tag="xpps", bufs=1,
                                         name="xpps")
                        for dh in range(DH):
                            nc.tensor.matmul(
                                xpps[:], wxp_t[:, dh * 40:(dh + 1) * 40],
                                xcr[:, dh * T:(dh + 1) * T],
                                start=(dh == 0), stop=(dh == 1))
                        dt_sb = sb.tile([8, T], F32R, tag="dt_sb", name="dt_sb")
                        nc.scalar.copy(dt_sb[:], xpps[0:8, :])
                        bch_sb = sb.tile([40, T], BF16, tag="bch_sb",
                                         name="bch_sb")
                        nc.scalar.copy(bch_sb[:], xpps[:])
                        dtd = dram.tile([8, T], F32R, tag="dtd", name="dtd")
                        nc.sync.dma_start(dtd[:], dt_sb[:])
                        bcd = dram.tile([32, T], BF16, tag="bcd", name="bcd")
                        nc.sync.dma_start(bcd[:], bch_sb[8:40, :])
                        bcds.append((bcd, dtd))

                    carry = sb.tile([128, 32], F32, tag="carry", name="carry")
                    nc.gpsimd.memset(carry[:], 0.0)

                    def emit_B(ci, b=b, carry=carry):
                        t0 = ci * T
                        bcd, dtd = _bcds[ci]

                        def bc_tile(j):
                            t = bcp.tile([128, T], BF16, tag="bc", name="bc")
                            src = AP(tensor=bcd.tensor,
                                     offset=bcd.offset + j * T,
                                     ap=[[0, 128], [1, T]])
                            nc.sync.dma_start(t[:], src)
                            return t

                        dt8 = sb.tile([8, T], F32R, tag="dt8", name="dt8")
                        nc.sync.dma_start(dt8[:], dtd[:])
                        delta, du = {}, {}
                        for dh in range(DH):
                            dlps = psum.tile([128, T], F32, tag="dlps", bufs=2,
                                             name="dlps")
                            nc.tensor.matmul(
                                dlps[:], wdt_t[:, dh * 128:(dh + 1) * 128],
                                dt8[:], start=True, stop=True)
                            esb = sc.tile([128, T], F32, tag="esb", name="esb")
                            nc.scalar.activation(esb[:], dlps[:], ACTF.Exp,
                                                 bias=bdt_t[:, dh:dh + 1])
                            delta[dh] = sb.tile([128, T], F32, tag=f"dl{dh}",
                                                name=f"dl{dh}")
                            nc.scalar.activation(delta[dh][:], esb[:], ACTF.Ln,
                                                 bias=1.0)
                            du[dh] = sb.tile([128, T], BF16, tag=f"du{dh}",
                                             name=f"du{dh}")
                            nc.vector.tensor_tensor(du[dh][:], delta[dh][:],
                                                    xcf[dh][:, t0:t0 + T],
                                                    op=ALU.mult)
                        for dh in range(DH):
                            yy = psum.tile([128, T], F32, tag="yy", bufs=2,
                                           name="yy")
                            for n in range(DST):
                                j = dh * 16 + n
                                a_t = sc.tile([128, T], F32, tag="a", name="a")
                                nc.scalar.activation(
                                    a_t[:], delta[dh][:], ACTF.Exp,
                                    scale=acols_t[:, j:j + 1])
                                Bn = bc_tile(n)
                                bb = sc.tile([128, T], BF16, tag="bb", name="bb")
                                nc.vector.tensor_tensor(bb[:], du[dh][:], Bn[:],
                                                        op=ALU.mult)
                                h = sc.tile([128, T], HDT, tag="h", name="h")
                                nc.vector.tensor_tensor_scan(
                                    h[:], a_t[:], bb[:], carry[:, j:j + 1],
                                    op0=ALU.mult, op1=ALU.add)
                                nc.scalar.copy(carry[:, j:j + 1], h[:, T - 1:T])
                                Cn = bc_tile(16 + n)
                                p = sc.tile([128, T], BF16, tag="p", name="p")
                                nc.vector.tensor_tensor(p[:], h[:], Cn[:],
                                                        op=ALU.mult)
                                nc.tensor.matmul(yy[:], ident_t[:], p[:],
                                                 start=(n == 0), stop=(n == 15))
                            t1 = sc.tile([128, T], F32, tag="t1", name="t1")
                            nc.vector.scalar_tensor_tensor(
                                t1[:], xcf[dh][:, t0:t0 + T],
                                dpar_t[:, dh:dh + 1], yy[:],
                                op0=ALU.mult, op1=ALU.add)
                            nc.vector.tensor_tensor(yg[dh][:, t0:t0 + T],
                                                    t1[:],
                                                    szf[dh][:, t0:t0 + T],
                                                    op=ALU.mult)

                    # software pipeline: B lags A by 2 chunks
                    _bcds = bcds
                    emit_A(0)
                    for ci in range(NCH):
                        if ci + 1 < NCH:
                            emit_A(ci + 1)
                        emit_B(ci)

                    # out-projection with folded (h,w) transpose,
                    # split by hh-halves so the first half (depends only on
                    # t < 2048) overlaps the tail of phase B
                    for hh0 in (0, 32):
                        for wc in range(8):
                            ops = psum.tile([128, 256], F32, tag="ops", bufs=1,
                                            name="ops")
                            for dh in range(DH):
                                rhs = yg[dh].rearrange("p (h w) -> p w h", w=64)
                                rhs = rhs[:, wc * 8:(wc + 1) * 8,
                                          hh0:hh0 + 32]
                                nc.tensor.matmul(
                                    ops[:], wout_t[:, dh * 128:(dh + 1) * 128],
                                    rhs, start=(dh == 0), stop=(dh == 1))
                            osb = sc.tile([128, 256], F32, tag="osb",
                                          name="osb")
                            nc.scalar.copy(osb[:], ops[:])
                            nc.sync.dma_start(
                                out_d.ap()[b][:, wc * 8:(wc + 1) * 8,
                                              hh0:hh0 + 32], osb[:])

    nc.compile()
    return nc


def _prep_inputs(x, W_in, conv_w, conv_b, W_xproj, W_dt, b_dt, A_log,
                 D_param, W_out):
    W2 = (W_in[:, :256][:, :, None] * conv_w[None, :, :])
    w2k = np.ascontiguousarray(W2.transpose(2, 0, 1)).astype(np.float32)
    winz = np.ascontiguousarray(W_in[:, 256:]).astype(np.float32)
    wxp = np.ascontiguousarray(W_xproj.reshape(2, 128, 40)).astype(np.float32)
    wdt = np.ascontiguousarray(W_dt).astype(np.float32)
    wout = np.ascontiguousarray(W_out.reshape(2, 128, 128)).astype(np.float32)
    cb = np.ascontiguousarray(conv_b.reshape(2, 128).T).astype(np.float32)
    bdt = np.ascontiguousarray(b_dt.reshape(2, 128).T).astype(np.float32)
    dpar = np.ascontiguousarray(D_param.reshape(2, 128).T).astype(np.float32)
    A = -np.exp(A_log.astype(np.float64)).astype(np.float32)
    acols = np.ascontiguousarray(
        A.reshape(2, 128, 16).transpose(1, 0, 2).reshape(128, 32)).astype(np.float32)
    ident = np.eye(128, dtype=np.float32).astype(
        np.dtype("uint16") if False else None)
    import ml_dtypes
    ident = np.eye(128).astype(ml_dtypes.bfloat16)
    shared = dict(w2k=w2k, winz=winz, wxp=wxp, wdt=wdt, wout=wout,
                  cb=cb, bdt=bdt, dpar=dpar, acols=acols, ident=ident)
    xr = np.ascontiguousarray(np.asarray(x).reshape(B_GLOB, 128, L)).astype(np.float32)
    in_maps = []
    for c in range(N_CORES):
        m = dict(shared)
        m["x"] = np.ascontiguousarray(xr[c * B_LOC:(c + 1) * B_LOC])
        in_maps.append(m)
    return in_maps


def run(nc, in_maps):
    from concourse.bass_utils import run_bass_kernel_spmd
    res = run_bass_kernel_spmd(nc, in_maps, core_ids=list(range(N_CORES)))
    return np.concatenate([res.results[c]["out"] for c in range(N_CORES)], axis=0)


def kernel(**inputs):
    if "nc" not in _BUILT:
        _BUILT["nc"] = build_module()
    in_maps = _prep_inputs(**{k: np.asarray(v) for k, v in inputs.items()})
    return run(_BUILT["nc"], in_maps)


if __name__ == "__main__":
    data = np.load("/root/problem/ref_cache.npz")
    inputs = {k: data[k] for k in data.files if k != "out"}
    out = kernel(**inputs)
    ref = data["out"]
    err = np.abs(out - ref).max() / np.abs(ref).max()
    rel = np.linalg.norm(out - ref) / np.linalg.norm(ref)
    print(f"max-abs/ref-max: {err:.3e}   fro rel: {rel:.3e}")

